# revision 1
# baseline (speedup 1.0000x reference)
"""AAGNN GraphConvolution kernel for 8 Trainium2 NeuronCores.

Computes relu(degree_norm * (adj @ (x @ W)) + b) for
x[16384,128], adj[16384,16384], degree_norm[16384,1], W[128,64], b[64].

Sharding: 1D row partition of the output nodes across 8 cores (2048 rows
each). Each core receives the transposed row-block of the adjacency
(adjT[16384, 2048], contiguous) so the TensorEngine can contract over the
full node axis with contiguous DMA, plus replicated xT/W/b and its
degree_norm slice. No cross-core communication is needed.

Device program per core:
  support = x @ W               (redundant on every core, [16384, 64] in SBUF)
  aggT    = support.T-weighted stream of adjT  -> PSUM [64, 2048]
  out     = relu(deg * aggT + b)               -> DRAM [64, 2048]
Host transposes/concats the per-core outputs back to [16384, 64].
"""

import sys

if "/opt/trn_rl_repo" not in sys.path:
    sys.path.insert(0, "/opt/trn_rl_repo")

import numpy as np
import ml_dtypes

import concourse.bass as bass  # noqa: F401  (AP helpers)
import concourse.mybir as mybir
import concourse.tile as tile
from concourse import bacc
from concourse.bass_utils import run_bass_kernel_spmd

N_NODES = 16384
F = 128  # feature size
H = 64  # hidden size
N_CORES = 8
ROWS = N_NODES // N_CORES  # 2048 output rows per core
KB = 128  # contraction block (partition dim)

# Tunables
USE_BF16 = True  # stream adjacency + support in bf16 (fp32 accumulation)
ADJ_BUFS = 6  # in-flight adjacency DMA tiles
KB_PER_TILE = 2  # k-blocks per adjacency DMA (tile bytes = KB_PER_TILE * ROWS * elt)


def build_nc(
    n_nodes: int = N_NODES,
    rows: int = ROWS,
    use_bf16: bool = USE_BF16,
    adj_bufs: int = ADJ_BUFS,
    kb_per_tile: int = KB_PER_TILE,
):
    """Build the single-core Bass program (same program on every core)."""
    f32 = mybir.dt.float32
    adt = mybir.dt.bfloat16 if use_bf16 else f32
    nkb = n_nodes // KB  # number of contraction blocks
    n_slice = min(512, rows)  # matmul moving free dim / psum bank slice
    n_acc = rows // n_slice  # psum accumulators
    x_chunk = min(2048, n_nodes)  # xT streamed in chunks of this many nodes
    n_xc = n_nodes // x_chunk

    nc = bacc.Bacc("TRN2", debug=False, num_devices=N_CORES)
    adjT = nc.declare_dram_parameter("adjT", [n_nodes, rows], adt, isOutput=False)
    xT = nc.declare_dram_parameter("xT", [F, n_nodes], f32, isOutput=False)
    Wp = nc.declare_dram_parameter("W", [F, H], f32, isOutput=False)
    bp = nc.declare_dram_parameter("b", [H, 1], f32, isOutput=False)
    degp = nc.declare_dram_parameter("deg", [1, rows], f32, isOutput=False)
    outp = nc.declare_dram_parameter("out", [H, rows], f32, isOutput=True)

    with tile.TileContext(nc) as tc:
        with (
            tc.tile_pool(name="const", bufs=1) as cpool,
            tc.tile_pool(name="xc", bufs=2) as xpool,
            tc.tile_pool(name="adj", bufs=adj_bufs) as apool,
            tc.tile_pool(name="spsum", bufs=2, space="PSUM") as spool,
            tc.tile_pool(name="accs", bufs=1, space="PSUM") as accpool,
            tc.tile_pool(name="epi", bufs=2) as epool,
        ):
            # ---- constants ----
            w_sb = cpool.tile([F, H], f32, tag="w")
            nc.sync.dma_start(out=w_sb[:], in_=Wp[:, :])
            b_sb = cpool.tile([H, 1], f32, tag="b")
            nc.sync.dma_start(out=b_sb[:], in_=bp[:, :])
            deg_bc = cpool.tile([H, rows], f32, tag="deg")
            nc.sync.dma_start(out=deg_bc[:], in_=degp[:, :].to_broadcast([H, rows]))

            # ---- support = x @ W, stored [k partitions, h free] per k-block ----
            support_sb = cpool.tile([KB, nkb * H], adt, tag="support")
            for c in range(n_xc):
                xc = xpool.tile([F, x_chunk], f32, tag="xc")
                nc.sync.dma_start(
                    out=xc[:], in_=xT[:, c * x_chunk : (c + 1) * x_chunk]
                )
                for i in range(x_chunk // KB):
                    kb = c * (x_chunk // KB) + i
                    ps = spool.tile([KB, H], f32, tag="spsum")
                    nc.tensor.matmul(
                        out=ps[:],
                        lhsT=xc[:, i * KB : (i + 1) * KB],
                        rhs=w_sb[:],
                        start=True,
                        stop=True,
                    )
                    nc.vector.tensor_copy(
                        out=support_sb[:, kb * H : (kb + 1) * H], in_=ps[:]
                    )

            # ---- aggregation: aggT[h, m] += support_kb.T-stationary @ adjT ----
            accs = [
                accpool.tile([H, n_slice], f32, tag=f"acc{m}", name=f"acc{m}")
                for m in range(n_acc)
            ]
            n_tiles = nkb // kb_per_tile
            for t in range(n_tiles):
                a = apool.tile([KB, kb_per_tile * rows], adt, tag="adj", name="a")
                if kb_per_tile == 1:
                    nc.sync.dma_start(
                        out=a[:], in_=adjT[t * KB : (t + 1) * KB, :]
                    )
                else:
                    nc.sync.dma_start(
                        out=a[:].rearrange("p (g m) -> p g m", g=kb_per_tile),
                        in_=adjT[
                            t * kb_per_tile * KB : (t + 1) * kb_per_tile * KB, :
                        ].rearrange("(g p) m -> p g m", p=KB),
                    )
                for j in range(kb_per_tile):
                    kb = t * kb_per_tile + j
                    for m in range(n_acc):
                        nc.tensor.matmul(
                            out=accs[m][:],
                            lhsT=support_sb[:, kb * H : (kb + 1) * H],
                            rhs=a[:, j * rows + m * n_slice : j * rows + (m + 1) * n_slice],
                            start=(kb == 0),
                            stop=(kb == nkb - 1),
                        )

            # ---- epilogue: relu(deg * aggT + b) ----
            o_sb = epool.tile([H, rows], f32, tag="o", name="o")
            for m in range(n_acc):
                tmp = epool.tile([H, n_slice], f32, tag="tmp", name="tmp")
                nc.vector.tensor_tensor(
                    out=tmp[:],
                    in0=accs[m][:],
                    in1=deg_bc[:, m * n_slice : (m + 1) * n_slice],
                    op=mybir.AluOpType.mult,
                )
                nc.scalar.activation(
                    out=o_sb[:, m * n_slice : (m + 1) * n_slice],
                    in_=tmp[:],
                    func=mybir.ActivationFunctionType.Relu,
                    bias=b_sb[:],
                )
            nc.sync.dma_start(out=outp[:, :], in_=o_sb[:])

    nc.compile()
    return nc


def make_in_maps(x, adj_matrix, degree_norm, W, b, use_bf16=USE_BF16):
    """Shard the full inputs into per-core input maps (host-side, numpy)."""
    adt = ml_dtypes.bfloat16 if use_bf16 else np.float32
    xT = np.ascontiguousarray(x.T, dtype=np.float32)
    Wf = np.ascontiguousarray(W, dtype=np.float32)
    bf = np.ascontiguousarray(b, dtype=np.float32).reshape(H, 1)
    in_maps = []
    for c in range(N_CORES):
        r0, r1 = c * ROWS, (c + 1) * ROWS
        adjT_c = np.ascontiguousarray(adj_matrix[r0:r1, :].T, dtype=adt)
        deg_c = np.ascontiguousarray(
            degree_norm[r0:r1].reshape(-1), dtype=np.float32
        ).reshape(1, ROWS)
        in_maps.append({"adjT": adjT_c, "xT": xT, "W": Wf, "b": bf, "deg": deg_c})
    return in_maps


_nc_cache = {}


def _get_nc():
    key = (USE_BF16, ADJ_BUFS, KB_PER_TILE)
    if key not in _nc_cache:
        _nc_cache[key] = build_nc()
    return _nc_cache[key]


def kernel(x, adj_matrix, degree_norm, W, b):
    x = np.asarray(x)
    adj_matrix = np.asarray(adj_matrix)
    degree_norm = np.asarray(degree_norm)
    W = np.asarray(W)
    b = np.asarray(b)

    nc = _get_nc()
    in_maps = make_in_maps(x, adj_matrix, degree_norm, W, b)
    res = run_bass_kernel_spmd(nc, in_maps, core_ids=list(range(N_CORES)))
    out = np.empty((N_NODES, H), dtype=np.float32)
    for c in range(N_CORES):
        out[c * ROWS : (c + 1) * ROWS, :] = res.results[c]["out"].T
    return out


# revision 7
# speedup vs baseline: 1.0900x; 1.0900x over previous
"""AAGNN GraphConvolution kernel for 8 Trainium2 NeuronCores.

Computes relu(degree_norm * (adj @ (x @ W)) + b) for
x[16384,128], adj[16384,16384], degree_norm[16384,1], W[128,64], b[64].

Sharding: 1D row partition of the output nodes across 8 cores (2048 rows
each). Each core receives the transposed row-block of the adjacency
(adjT[16384, 2048], contiguous) so the TensorEngine can contract over the
full node axis with contiguous DMA, plus replicated xT/W/b and its
degree_norm slice. No cross-core communication is needed.

Device program per core:
  support = x @ W               (redundant on every core, [16384, 64] in SBUF)
  aggT    = support.T-weighted stream of adjT  -> PSUM [64, 2048]
  out     = relu(deg * aggT + b)               -> DRAM [64, 2048]
Host transposes/concats the per-core outputs back to [16384, 64].
"""

import sys

if "/opt/trn_rl_repo" not in sys.path:
    sys.path.insert(0, "/opt/trn_rl_repo")

import numpy as np
import ml_dtypes

import concourse.bass as bass  # noqa: F401  (AP helpers)
import concourse.mybir as mybir
import concourse.tile as tile
from concourse import bacc
from concourse.bass_utils import run_bass_kernel_spmd

N_NODES = 16384
F = 128  # feature size
H = 64  # hidden size
N_CORES = 8
ROWS = N_NODES // N_CORES  # 2048 output rows per core
KB = 128  # contraction block (partition dim)

# Tunables
USE_BF16 = True  # stream adjacency/x/W in bf16 (fp32 accumulation)
ADJ_BUFS = 5  # in-flight adjacency DMA tiles
KB_PER_TILE = 4  # k-blocks per adjacency DMA (tile bytes = KB_PER_TILE * ROWS * elt)


def build_nc(
    n_nodes: int = N_NODES,
    rows: int = ROWS,
    use_bf16: bool = USE_BF16,
    adj_bufs: int = ADJ_BUFS,
    kb_per_tile: int = KB_PER_TILE,
):
    """Build the single-core Bass program (same program on every core)."""
    f32 = mybir.dt.float32
    adt = mybir.dt.bfloat16 if use_bf16 else f32
    nkb = n_nodes // KB  # number of contraction blocks
    n_slice = min(512, rows)  # matmul moving free dim / psum bank slice
    n_acc = rows // n_slice  # psum accumulators
    x_chunk = min(2048, n_nodes)  # xT streamed in chunks of this many nodes
    n_xc = n_nodes // x_chunk

    nc = bacc.Bacc("TRN2", debug=False, num_devices=N_CORES)
    adjT = nc.declare_dram_parameter("adjT", [n_nodes, rows], adt, isOutput=False)
    xT = nc.declare_dram_parameter("xT", [F, n_nodes], adt, isOutput=False)
    Wp = nc.declare_dram_parameter("W", [F, H], adt, isOutput=False)
    bp = nc.declare_dram_parameter("b", [H, 1], f32, isOutput=False)
    degp = nc.declare_dram_parameter("deg", [1, rows], f32, isOutput=False)
    outp = nc.declare_dram_parameter("out", [H, rows], f32, isOutput=True)

    with tile.TileContext(nc) as tc:
        with (
            tc.tile_pool(name="const", bufs=1) as cpool,
            tc.tile_pool(name="xc", bufs=2) as xpool,
            tc.tile_pool(name="adj", bufs=adj_bufs) as apool,
            tc.tile_pool(name="spsum", bufs=2, space="PSUM") as spool,
            tc.tile_pool(name="accs", bufs=1, space="PSUM") as accpool,
            tc.tile_pool(name="epi", bufs=2) as epool,
        ):
            # ---- constants ----
            w_sb = cpool.tile([F, H], adt, tag="w")
            nc.sync.dma_start(out=w_sb[:], in_=Wp[:, :])
            b_sb = cpool.tile([H, 1], f32, tag="b")
            nc.sync.dma_start(out=b_sb[:], in_=bp[:, :])
            deg_bc = cpool.tile([H, rows], f32, tag="deg")
            nc.sync.dma_start(out=deg_bc[:], in_=degp[:, :].to_broadcast([H, rows]))

            # ---- support = x @ W, stored [k partitions, h free] per k-block ----
            support_sb = cpool.tile([KB, nkb * H], adt, tag="support")
            for c in range(n_xc):
                xc = xpool.tile([F, x_chunk], adt, tag="xc")
                nc.sync.dma_start(
                    out=xc[:], in_=xT[:, c * x_chunk : (c + 1) * x_chunk]
                )
                for i in range(x_chunk // KB):
                    kb = c * (x_chunk // KB) + i
                    ps = spool.tile([KB, H], f32, tag="spsum")
                    nc.tensor.matmul(
                        out=ps[:],
                        lhsT=xc[:, i * KB : (i + 1) * KB],
                        rhs=w_sb[:],
                        start=True,
                        stop=True,
                    )
                    nc.vector.tensor_copy(
                        out=support_sb[:, kb * H : (kb + 1) * H], in_=ps[:]
                    )

            # ---- aggregation: aggT[h, m] += support_kb.T-stationary @ adjT ----
            accs = [
                accpool.tile([H, n_slice], f32, tag=f"acc{m}", name=f"acc{m}")
                for m in range(n_acc)
            ]
            n_tiles = nkb // kb_per_tile
            for t in range(n_tiles):
                a = apool.tile([KB, kb_per_tile * rows], adt, tag="adj", name="a")
                # adjacency streams on the ACT HWDGE ring so it is not
                # head-of-line blocked behind the xT/const DMAs on the SP ring
                if kb_per_tile == 1:
                    nc.scalar.dma_start(
                        out=a[:], in_=adjT[t * KB : (t + 1) * KB, :]
                    )
                else:
                    nc.scalar.dma_start(
                        out=a[:].rearrange("p (g m) -> p g m", g=kb_per_tile),
                        in_=adjT[
                            t * kb_per_tile * KB : (t + 1) * kb_per_tile * KB, :
                        ].rearrange("(g p) m -> p g m", p=KB),
                    )
                for j in range(kb_per_tile):
                    kb = t * kb_per_tile + j
                    for m in range(n_acc):
                        nc.tensor.matmul(
                            out=accs[m][:],
                            lhsT=support_sb[:, kb * H : (kb + 1) * H],
                            rhs=a[:, j * rows + m * n_slice : j * rows + (m + 1) * n_slice],
                            start=(kb == 0),
                            stop=(kb == nkb - 1),
                        )

            # ---- epilogue: relu(deg * aggT + b) ----
            o_sb = epool.tile([H, rows], f32, tag="o", name="o")
            for m in range(n_acc):
                tmp = epool.tile([H, n_slice], f32, tag="tmp", name="tmp")
                nc.vector.tensor_tensor(
                    out=tmp[:],
                    in0=accs[m][:],
                    in1=deg_bc[:, m * n_slice : (m + 1) * n_slice],
                    op=mybir.AluOpType.mult,
                )
                nc.scalar.activation(
                    out=o_sb[:, m * n_slice : (m + 1) * n_slice],
                    in_=tmp[:],
                    func=mybir.ActivationFunctionType.Relu,
                    bias=b_sb[:],
                )
            nc.sync.dma_start(out=outp[:, :], in_=o_sb[:])

    nc.compile()
    return nc


def make_in_maps(x, adj_matrix, degree_norm, W, b, use_bf16=USE_BF16):
    """Shard the full inputs into per-core input maps (host-side, numpy)."""
    adt = ml_dtypes.bfloat16 if use_bf16 else np.float32
    xT = np.ascontiguousarray(x.T, dtype=adt)
    Wf = np.ascontiguousarray(W, dtype=adt)
    bf = np.ascontiguousarray(b, dtype=np.float32).reshape(H, 1)
    in_maps = []
    for c in range(N_CORES):
        r0, r1 = c * ROWS, (c + 1) * ROWS
        adjT_c = np.ascontiguousarray(adj_matrix[r0:r1, :].T, dtype=adt)
        deg_c = np.ascontiguousarray(
            degree_norm[r0:r1].reshape(-1), dtype=np.float32
        ).reshape(1, ROWS)
        in_maps.append({"adjT": adjT_c, "xT": xT, "W": Wf, "b": bf, "deg": deg_c})
    return in_maps


_nc_cache = {}


def _get_nc():
    key = (USE_BF16, ADJ_BUFS, KB_PER_TILE)
    if key not in _nc_cache:
        _nc_cache[key] = build_nc()
    return _nc_cache[key]


def kernel(x, adj_matrix, degree_norm, W, b):
    x = np.asarray(x)
    adj_matrix = np.asarray(adj_matrix)
    degree_norm = np.asarray(degree_norm)
    W = np.asarray(W)
    b = np.asarray(b)

    nc = _get_nc()
    in_maps = make_in_maps(x, adj_matrix, degree_norm, W, b)
    res = run_bass_kernel_spmd(nc, in_maps, core_ids=list(range(N_CORES)))
    out = np.empty((N_NODES, H), dtype=np.float32)
    for c in range(N_CORES):
        out[c * ROWS : (c + 1) * ROWS, :] = res.results[c]["out"].T
    return out
